# revision 17
# baseline (speedup 1.0000x reference)
"""Trainium2 Bass kernel for nn_Net_4200478015619 (dense_mlp).

Computes, for x (262144, 128) fp32 and W (100, 128) fp32:
    z   = x @ W.T                      # (B, 100)
    y   = z**3 + 0.1 * z
    out = sum(y, axis=1, keepdims=True)  # (B, 1)

Sharding: pure data parallel over 8 NeuronCores — core c gets rows
[c*32768, (c+1)*32768). Each shard is transposed (and cast to bf16) on
the host to xT (128, 32768) so the feature dim lands on SBUF partitions
and the matmul contraction needs no on-chip transpose. bf16 operands
keep every matmul on the fast 1-cycle/column PE path and halve the HBM
traffic; fp32 PSUM accumulation plus fp32 DVE math keeps the end-to-end
error ~3e-3 of the output scale.

The cubic nonlinearity runs as ONE custom DVE micro-op program
(registered at build time into concourse's per-NEFF DVE table):
    CUBE_AXPB_ANT: out = (Src0^2 + c0) * Src0    # = z^3 + alpha*z

Per-core dataflow (Tile-scheduled pipeline, 32 z-tiles of 1024 rows):
    DMA    : xT tiles [128, 2048] bf16 -> SBUF
    PE MM1 : zT [100, 1024] = W @ xT_chunk     (2x N=512 bf16 matmuls)
    DVE    : y = (zT^2 + 0.1) * zT   PSUM -> bf16 SBUF (one 1024-wide op)
    PE MM2 : sel_j.T @ y (bf16) accumulates chunk j's head-sums into row
             j of a [16, 512] PSUM tile (selector columns, PSUM accumulate)
    ACT    : copy [16, 512] PSUM -> SBUF (once per 16 chunks)
    DMA    : [16, 512] SBUF -> DRAM

A short burst of dummy bf16 matmuls at kernel start keeps the PE's HAM
clock-gate warm through the DMA pipeline-fill so real matmuls run at
2.4 GHz from the first tile.
"""

import numpy as np

import concourse.bacc as bacc
import concourse.mybir as mybir
import concourse.tile as tile
from concourse.bass_utils import run_bass_kernel_spmd

# --- TileContext exit-drain legalization -----------------------------------
# This toolchain's walrus caps CTRL-class instructions at ONE sync wait; the
# stock TileContext exit drain carries one wait per logical proc and fails
# codegen. Split the waits across per-engine single-wait NOPs instead.
from concourse.vector_clock import ScopedClock, VectorClock


def _patched_drain_and_barrier(self, tick_clock, wait_clock):
    g = tick_clock.global_clock
    n = len(g)
    pending = [i for i in range(n) if g[i] > 0]
    engines = [e for e in self.nc.engines.values()]
    for k, p in enumerate(pending):
        vec = [0] * n
        vec[p] = g[p]
        eng = engines[k % len(engines)]
        nop_inst = eng.nop()
        wait_clock.add_sem_waits(nop_inst.ins, ScopedClock({None: VectorClock(vec)}))
    # The all-engine barrier joins the per-engine wait-NOPs, so the drain
    # itself needs no waits (each engine's NOPs precede its barrier entry
    # in FIFO order).
    self.nc.sync.drain()
    self.nc.all_engine_barrier()
    assert self.sems is not None
    popped = self.nc._tile_sem_poison_stack.pop()
    assert popped is self._sem_poison
    self.nc.clear_and_free_semaphores(list(self.sems.allocated().values()))
    self.nc.all_engine_barrier()


tile.TileContext._drain_and_barrier = _patched_drain_and_barrier
# ---------------------------------------------------------------------------


N_CORES = 8
B = 262144
B_CORE = B // N_CORES  # 32768
F = 128
M = 100
ALPHA = 0.1
CHUNK = 512          # matmul moving-dim tile (one PSUM bank)
ZTILE = 1024         # z-tile width (2 chunks)
DTILE = 2048         # input DMA tile width
N_DTILES = B_CORE // DTILE  # 16
N_CHUNKS = B_CORE // CHUNK  # 64
GROUP = 16           # chunks accumulated per output PSUM tile
N_WARMUP = 8        # dummy matmuls to hold HAM warm through pipeline fill

_CUBE_OP = None


def _register_cube_op():
    """Register `out = (Src0^2 + c0) * Src0` as a custom DVE op so the whole
    cubic runs as one Vector instruction straight out of PSUM."""
    global _CUBE_OP
    if _CUBE_OP is not None:
        return _CUBE_OP
    import concourse.dve_ops as dve_ops
    from concourse.dve_spec import Spec, Src0, C0, sq, lower
    from concourse.dve_uop import DveOpSpec

    name = "CUBE_AXPB_ANT"
    for op in dve_ops.OPS:
        if op.name == name:
            _CUBE_OP = op
            return op
    spec = Spec(
        body=(sq(Src0) + C0) * Src0,
        reference=lambda in0, in1, s0, s1, imm2: (
            (in0.astype(np.float32) ** 2 + s0) * in0.astype(np.float32)
        ).astype(np.float32),
    )
    row = dve_ops._CUSTOM_DVE_ROW_BASE + len(dve_ops.OPS)
    assert row < 0x20, "custom-DVE opcode rows exhausted"
    shas = {
        ver: DveOpSpec(
            name=name, opcode=row, uops=lower(spec, ver=ver), rd1_en=False
        ).sha(ver)
        for ver in ("v3", "v4")
    }
    op = dve_ops.DveOp(name, spec, subdim=False, uops_sha=shas)
    dve_ops.OPS.append(op)
    dve_ops._SUB_OPCODE_FOR_NAME[name] = row
    dve_ops.CUSTOM_DVE_SPECS[name] = spec
    _CUBE_OP = op
    return op


def build_nc():
    cube_op = _register_cube_op()
    nc = bacc.Bacc()
    xt = nc.declare_dram_parameter("xt", [F, B_CORE], mybir.dt.bfloat16, isOutput=False)
    wt = nc.declare_dram_parameter("wt", [F, M], mybir.dt.bfloat16, isOutput=False)
    # sel[:, GROUP*j + i] = (i == j): per-chunk selector stationaries so the
    # GROUP per-chunk head-sum matmuls accumulate into distinct rows of one
    # [GROUP, CHUNK] PSUM tile.
    sel = nc.declare_dram_parameter(
        "sel", [M, GROUP * GROUP], mybir.dt.bfloat16, isOutput=False
    )
    out = nc.declare_dram_parameter(
        "out", [N_CHUNKS, CHUNK], mybir.dt.float32, isOutput=True
    )

    with tile.TileContext(nc) as tc:
        with (
            tc.tile_pool(name="wpool", bufs=1) as wpool,
            tc.tile_pool(name="xpool", bufs=4) as xpool,
            tc.tile_pool(name="ypool", bufs=3) as ypool,
            tc.tile_pool(name="opool", bufs=2) as opool,
            tc.tile_pool(name="zpsum", bufs=3, space="PSUM") as zpsum,
            tc.tile_pool(name="opsum", bufs=2, space="PSUM") as opsum,
        ):
            ws = wpool.tile([F, M], mybir.dt.bfloat16)
            nc.sync.dma_start(out=ws[:], in_=wt[:])
            sel_s = wpool.tile([M, GROUP * GROUP], mybir.dt.bfloat16)
            nc.sync.dma_start(out=sel_s[:], in_=sel[:])

            # HAM warm-up: keep the PE busy from t=0 so the 2.4 GHz clock
            # gate is open when the first real matmul issues.
            wu_w = wpool.tile([F, F], mybir.dt.bfloat16)
            nc.gpsimd.memset(wu_w[:], 0.0)
            wu_x = wpool.tile([F, CHUNK], mybir.dt.bfloat16)
            nc.gpsimd.memset(wu_x[:], 0.0)
            wu_p = zpsum.tile([M, ZTILE], mybir.dt.float32, tag="zt")
            for _ in range(N_WARMUP):
                nc.tensor.matmul(
                    wu_p[:, :CHUNK], lhsT=wu_w[:, :M], rhs=wu_x[:], start=True, stop=True
                )

            # Input tile plan: a small first tile so the pipeline fills
            # fast, then steady 2048-wide tiles. Everything on one HWDGE
            # queue -- the steady state is Vector-bound, so peak DMA rate
            # matters less than first-tile latency.
            widths = [1024, 1024] + [2048] * ((B_CORE - 2048) // 2048)
            assert sum(widths) == B_CORE
            col = 0
            for dt_i, width in enumerate(widths):
                xs = xpool.tile([F, width], mybir.dt.bfloat16, tag="xs")
                # the two pipeline-fill tiles go on the SP queue (its first
                # transfer completes soonest); the steady stream alternates
                # queues for bandwidth
                dma_eng = nc.sync if (dt_i < 2 or dt_i % 2 == 0) else nc.scalar
                dma_eng.dma_start(out=xs[:], in_=xt[:, col : col + width])
                for zt_i in range(width // ZTILE):
                    t = (col // ZTILE) + zt_i  # z-tile index, 0..31
                    zt = zpsum.tile([M, ZTILE], mybir.dt.float32, tag="zt")
                    for h in range(ZTILE // CHUNK):
                        nc.tensor.matmul(
                            zt[:, h * CHUNK : (h + 1) * CHUNK],
                            lhsT=ws[:],
                            rhs=xs[
                                :,
                                zt_i * ZTILE
                                + h * CHUNK : zt_i * ZTILE
                                + (h + 1) * CHUNK,
                            ],
                            start=True,
                            stop=True,
                        )
                    y = ypool.tile([M, ZTILE], mybir.dt.bfloat16)
                    nc.vector._custom_dve(cube_op, out=y[:], in0=zt[:], s0=ALPHA)
                    # chunks 2t, 2t+1 accumulate into o_acc rows j, j+1
                    if (2 * t) % GROUP == 0:
                        o_acc = opsum.tile([GROUP, CHUNK], mybir.dt.float32)
                    for h in range(ZTILE // CHUNK):
                        j = (2 * t + h) % GROUP
                        nc.tensor.matmul(
                            o_acc[:],
                            lhsT=sel_s[:, GROUP * j : GROUP * (j + 1)],
                            rhs=y[:, h * CHUNK : (h + 1) * CHUNK],
                            start=(j == 0),
                            stop=(j == GROUP - 1),
                        )
                    if (2 * t + 2) % GROUP == 0:
                        g = (2 * t + 1) // GROUP
                        osb = opool.tile([GROUP, CHUNK], mybir.dt.float32)
                        nc.scalar.copy(osb[:], o_acc[:])
                        nc.sync.dma_start(
                            out=out[g * GROUP : (g + 1) * GROUP, :], in_=osb[:]
                        )
                col += width
    nc.finalize()
    return nc


def _run(x, W, trace=False, **run_kwargs):
    import ml_dtypes

    x = np.ascontiguousarray(x, dtype=np.float32)
    W = np.ascontiguousarray(W, dtype=np.float32)
    wt_np = np.ascontiguousarray(W.T.astype(ml_dtypes.bfloat16))  # (128, 100)

    sel_np = np.zeros((M, GROUP * GROUP), dtype=ml_dtypes.bfloat16)
    for j in range(GROUP):
        sel_np[:, GROUP * j + j] = 1.0

    in_maps = []
    for c in range(N_CORES):
        shard = x[c * B_CORE : (c + 1) * B_CORE, :]  # (32768, 128)
        xt_np = np.ascontiguousarray(shard.T.astype(ml_dtypes.bfloat16))
        in_maps.append({"xt": xt_np, "wt": wt_np, "sel": sel_np})

    nc = build_nc()
    res = run_bass_kernel_spmd(
        nc, in_maps, list(range(N_CORES)), trace=trace, **run_kwargs
    )
    outs = [res.results[c]["out"].reshape(B_CORE, 1) for c in range(N_CORES)]
    full = np.concatenate(outs, axis=0)  # (262144, 1)
    return full, res


def kernel(x, W):
    full, _ = _run(x, W)
    return full
